# revision 6
# baseline (speedup 1.0000x reference)
"""Trainium2 kernel for nn_Net_11312943858306.

Strategy: the FC head (~95% of FLOPs: 4x8192 points through
1016->512->512->1024, cross-camera max/mean reduce, 2048->512->128->2)
runs as a Bass/Tile SPMD kernel on 8 NeuronCores, data-parallel over
points (1024 points/core, all 4 cameras of a point on the same core ->
no collectives). The conv pyramid + bilinear gather (~5% of FLOPs,
data-dependent indexing) is prepared host-side in numpy.

Device layout: activations are [features-on-partitions, points-on-free];
weights are pre-transposed lhsT [in,out] bf16; matmuls accumulate fp32 in
PSUM; bias+relu fused on the scalar engine. Per core, points are split in
4 blocks of 256 (x 4 cams = 1024 columns) so SBUF holds weights + double
buffered activations.
"""

import numpy as np
import ml_dtypes

N_CAM = 4
N_PTS = 8192
IMG = 512
N_CORES = 8
PTS_PER_CORE = N_PTS // N_CORES          # 1024
BLK_PTS = 256                            # points per device block
N_BLK = PTS_PER_CORE // BLK_PTS          # 4
COLS = PTS_PER_CORE * N_CAM              # 4096
FEAT = 1016
FEAT_PAD = 1024

_compiled = None
LAST_RESULT = None


# ----------------------------------------------------------------------
# Host-side reference-exact feature extraction (conv pyramid + gather)
# ----------------------------------------------------------------------

def _conv3x3(x, w, b):
    # x [N,C,H,W] fp32, w [O,I,3,3], pad 1
    n, c, h, ww = x.shape
    o = w.shape[0]
    xp = np.pad(x, ((0, 0), (0, 0), (1, 1), (1, 1)))
    win = np.lib.stride_tricks.sliding_window_view(xp, (3, 3), axis=(2, 3))
    # win [N,C,H,W,3,3] -> [N,H,W,C*9]
    win = win.transpose(0, 2, 3, 1, 4, 5).reshape(n * h * ww, c * 9)
    wm = w.reshape(o, c * 9).T.astype(np.float32)  # [C*9, O]
    y = win.astype(np.float32) @ wm
    y = y.reshape(n, h, ww, o).transpose(0, 3, 1, 2)
    return y + b[None, :, None, None]


def _bn(x, gamma, beta, eps=1e-5):
    mu = x.mean(axis=(0, 2, 3), keepdims=True, dtype=np.float32)
    var = ((x - mu) ** 2).mean(axis=(0, 2, 3), keepdims=True, dtype=np.float32)
    xhat = (x - mu) / np.sqrt(var + eps)
    return gamma[None, :, None, None] * xhat + beta[None, :, None, None]


def _maxpool2(x):
    n, c, h, w = x.shape
    return x.reshape(n, c, h // 2, 2, w // 2, 2).max(axis=(3, 5))


def _bilinear_gather(x, px, py):
    H, W = x.shape[2], x.shape[3]
    px = px.astype(np.float32)
    py = py.astype(np.float32)
    xc = np.ceil(px)
    xf = xc - 1.0
    yc = np.ceil(py)
    yf = yc - 1.0
    inside = ((px <= H - 1) & (px >= 0) & (py <= W - 1) & (py >= 0)).astype(px.dtype)
    xc, xf, yc, yf = xc * inside, xf * inside, yc * inside, yf * inside
    cam = np.arange(x.shape[0])[:, None]
    xic, xif = xc.astype(np.int32), xf.astype(np.int32)
    yic, yif = yc.astype(np.int32), yf.astype(np.int32)
    v_cc = x[cam, :, xic, yic]
    v_fc = x[cam, :, xif, yic]
    v_cf = x[cam, :, xic, yif]
    v_ff = x[cam, :, xif, yif]
    w_cc = ((xf - px) * (yf - py))[..., None]
    w_fc = ((px - xc) * (yf - py))[..., None]
    w_cf = ((xf - px) * (py - yc))[..., None]
    w_ff = ((px - xc) * (py - yc))[..., None]
    return v_cc * w_cc + v_fc * w_fc + v_cf * w_cf + v_ff * w_ff


def _host_features(photo, projection_points, params):
    convs = [(np.asarray(w, np.float32), np.asarray(b, np.float32))
             for (w, b) in params["conv"]]
    bns = [(np.asarray(g, np.float32), np.asarray(b, np.float32))
           for (g, b) in params["bn"]]
    px = np.asarray(projection_points[:, :, 0], np.float32)
    py = np.asarray(projection_points[:, :, 1], np.float32)
    feats = []
    x = _bn(_conv3x3(np.asarray(photo, np.float32), *convs[0]), *bns[0])
    feats.append(_bilinear_gather(x, px, py))
    px, py = px / 2, py / 2
    ci = 1
    for _ in range(6):
        x = _maxpool2(x)
        x = np.maximum(_bn(_conv3x3(x, *convs[ci]), *bns[ci]), 0.0); ci += 1
        x = np.maximum(_bn(_conv3x3(x, *convs[ci]), *bns[ci]), 0.0); ci += 1
        feats.append(_bilinear_gather(x, px, py))
        px, py = px / 2, py / 2
    return np.concatenate(feats, axis=-1).astype(np.float32)  # [cam, N, 1016]


# ----------------------------------------------------------------------
# Device kernel (Bass/Tile), SPMD over 8 cores
# ----------------------------------------------------------------------

def _build_kernel():
    import concourse.bass as bass
    import concourse.bacc as bacc
    import concourse.mybir as mybir
    import concourse.tile as tile

    bf16 = mybir.dt.bfloat16
    f32 = mybir.dt.float32
    RELU = mybir.ActivationFunctionType.Relu
    IDENT = mybir.ActivationFunctionType.Identity
    MAX = mybir.AluOpType.max

    nc = bacc.Bacc()
    x_d = nc.dram_tensor("x", [FEAT_PAD, COLS], bf16, kind="ExternalInput")
    w0_d = nc.dram_tensor("w0", [1024, 512], bf16, kind="ExternalInput")
    w1_d = nc.dram_tensor("w1", [512, 512], bf16, kind="ExternalInput")
    w2_d = nc.dram_tensor("w2", [512, 1024], bf16, kind="ExternalInput")
    w3_d = nc.dram_tensor("w3", [2048, 512], bf16, kind="ExternalInput")
    w4_d = nc.dram_tensor("w4", [512, 128], bf16, kind="ExternalInput")
    w5_d = nc.dram_tensor("w5", [128, 32], bf16, kind="ExternalInput")
    ba_d = nc.dram_tensor("ba", [128, 21], f32, kind="ExternalInput")
    b5_d = nc.dram_tensor("b5", [32, 1], f32, kind="ExternalInput")
    out_d = nc.dram_tensor("out", [32, COLS // N_CAM], f32, kind="ExternalOutput")

    x_r = x_d[:, :].rearrange("(kc p) n -> kc p n", p=128)          # [8,128,4096]
    w_r = {
        0: w0_d[:, :].rearrange("(kc p) m -> kc p m", p=128),        # [8,128,512]
        1: w1_d[:, :].rearrange("(kc p) m -> kc p m", p=128),        # [4,128,512]
        2: w2_d[:, :].rearrange("(kc p) m -> kc p m", p=128),        # [4,128,1024]
        3: w3_d[:, :].rearrange("(kc p) m -> kc p m", p=128),        # [16,128,512]
        4: w4_d[:, :].rearrange("(kc p) m -> kc p m", p=128),        # [4,128,128]
    }

    with tile.TileContext(nc) as tc:
        with (
            tc.tile_pool(name="wpool", bufs=1) as wp,
            tc.tile_pool(name="xpool", bufs=2) as xp,
            tc.tile_pool(name="hpool", bufs=2) as hp,
            tc.tile_pool(name="opool", bufs=1) as op,
            tc.tile_pool(name="psum", bufs=2, space="PSUM") as pp,
        ):
            # resident weights/biases (one DMA per tensor)
            wdims = {0: (8, 512), 1: (4, 512), 2: (4, 1024), 3: (16, 512), 4: (4, 128)}
            wsrc = {0: w0_d, 1: w1_d, 2: w2_d, 3: w3_d, 4: w4_d}
            wt = {}
            for li, (nk, m) in wdims.items():
                t = wp.tile([128, nk * m], bf16, tag=f"wl{li}", name=f"wl{li}")
                nc.sync.dma_start(
                    out=t[:].rearrange("p (kc m) -> p kc m", m=m),
                    in_=wsrc[li][:, :].rearrange("(kc p) m -> p kc m", p=128))
                wt[li] = [t[:, k * m:(k + 1) * m] for k in range(nk)]
            w5t = wp.tile([128, 32], bf16, tag="w5")
            nc.sync.dma_start(out=w5t[:], in_=w5_d[:, :])
            bat = wp.tile([128, 21], f32, tag="ba")
            nc.sync.dma_start(out=bat[:], in_=ba_d[:, :])
            b5t = wp.tile([32, 1], f32, tag="b5t")
            nc.sync.dma_start(out=b5t[:], in_=b5_d[:, :])
            bt = {"b0": bat[:, 0:4], "b1": bat[:, 4:8], "b2": bat[:, 8:16],
                  "b3": bat[:, 16:20], "b4": bat[:, 20:21], "b5": b5t}

            out_sb = op.tile([32, COLS // N_CAM], f32, tag="out_sb")

            for b in range(N_BLK):
                c0 = b * 1024
                # load x block: one DMA, 8 K-chunks side by side
                xbt = xp.tile([128, 8192], bf16, tag="xbt", name="xbt")
                nc.sync.dma_start(
                    out=xbt[:].rearrange("p (kc n) -> p kc n", n=1024),
                    in_=x_d[:, :].rearrange("(kc p) n -> p kc n", p=128)[:, :, c0:c0 + 1024])
                xb = [xbt[:, k * 1024:(k + 1) * 1024] for k in range(8)]

                # fc0: 1024 -> 512, relu
                h0 = [hp.tile([128, 1024], bf16, tag=f"h0_{m}", name=f"h0_{m}") for m in range(4)]
                for m in range(4):
                    for nt in range(2):
                        ps = pp.tile([128, 512], f32, tag="ps_a")
                        for k in range(8):
                            nc.tensor.matmul(ps[:], wt[0][k][:, m * 128:(m + 1) * 128],
                                             xb[k][:, nt * 512:(nt + 1) * 512],
                                             start=(k == 0), stop=(k == 7))
                        nc.scalar.activation(h0[m][:, nt * 512:(nt + 1) * 512], ps[:],
                                             RELU, bias=bt["b0"][:, m:m + 1])

                # fc1: 512 -> 512, relu
                h1 = [hp.tile([128, 1024], bf16, tag=f"h1_{m}", name=f"h1_{m}") for m in range(4)]
                for m in range(4):
                    for nt in range(2):
                        ps = pp.tile([128, 512], f32, tag="ps_a")
                        for k in range(4):
                            nc.tensor.matmul(ps[:], wt[1][k][:, m * 128:(m + 1) * 128],
                                             h0[k][:, nt * 512:(nt + 1) * 512],
                                             start=(k == 0), stop=(k == 3))
                        nc.scalar.activation(h1[m][:, nt * 512:(nt + 1) * 512], ps[:],
                                             RELU, bias=bt["b1"][:, m:m + 1])

                # fc2: 512 -> 1024, no relu
                h2 = [hp.tile([128, 1024], bf16, tag=f"h2_{m}", name=f"h2_{m}") for m in range(8)]
                for m in range(8):
                    for nt in range(2):
                        ps = pp.tile([128, 512], f32, tag="ps_a")
                        for k in range(4):
                            nc.tensor.matmul(ps[:], wt[2][k][:, m * 128:(m + 1) * 128],
                                             h1[k][:, nt * 512:(nt + 1) * 512],
                                             start=(k == 0), stop=(k == 3))
                        nc.scalar.activation(h2[m][:, nt * 512:(nt + 1) * 512], ps[:],
                                             IDENT, bias=bt["b2"][:, m:m + 1])

                # cross-camera reduce: cols are [cam0|cam1|cam2|cam3] x 256 pts
                zc = []
                for m in range(8):
                    zm = hp.tile([128, 256], bf16, tag=f"zmx_{m}", name=f"zmx_{m}")
                    t01 = hp.tile([128, 256], bf16, tag=f"t01_{m}", name=f"t01_{m}")
                    nc.vector.tensor_tensor(out=t01[:], in0=h2[m][:, 0:256],
                                            in1=h2[m][:, 256:512], op=MAX)
                    nc.vector.tensor_tensor(out=zm[:], in0=h2[m][:, 512:768],
                                            in1=h2[m][:, 768:1024], op=MAX)
                    nc.vector.tensor_tensor(out=zm[:], in0=zm[:], in1=t01[:], op=MAX)
                    zc.append(zm)
                for m in range(8):
                    zs = hp.tile([128, 256], bf16, tag=f"zsm_{m}", name=f"zsm_{m}")
                    t01 = hp.tile([128, 256], bf16, tag=f"s01_{m}", name=f"s01_{m}")
                    nc.vector.tensor_add(out=t01[:], in0=h2[m][:, 0:256],
                                         in1=h2[m][:, 256:512])
                    nc.vector.tensor_add(out=zs[:], in0=h2[m][:, 512:768],
                                         in1=h2[m][:, 768:1024])
                    nc.vector.tensor_add(out=zs[:], in0=zs[:], in1=t01[:])
                    zc.append(zs)  # mean fold (x0.25) is baked into w3 rows

                # fc3: 2048 -> 512, relu   (N = 256)
                h3 = [hp.tile([128, 256], bf16, tag=f"h3_{m}", name=f"h3_{m}") for m in range(4)]
                for m in range(4):
                    ps = pp.tile([128, 256], f32, tag="ps_b")
                    for k in range(16):
                        nc.tensor.matmul(ps[:], wt[3][k][:, m * 128:(m + 1) * 128],
                                         zc[k][:], start=(k == 0), stop=(k == 15))
                    nc.scalar.activation(h3[m][:], ps[:], RELU,
                                         bias=bt["b3"][:, m:m + 1])

                # fc4: 512 -> 128, relu
                h4 = hp.tile([128, 256], bf16, tag="h4")
                ps = pp.tile([128, 256], f32, tag="ps_b")
                for k in range(4):
                    nc.tensor.matmul(ps[:], wt[4][k], h3[k][:],
                                     start=(k == 0), stop=(k == 3))
                nc.scalar.activation(h4[:], ps[:], RELU, bias=bt["b4"][:, 0:1])

                # fc5: 128 -> 2 (padded to 32)
                ps5 = pp.tile([32, 256], f32, tag="ps_c")
                nc.tensor.matmul(ps5[:], w5t[:], h4[:], start=True, stop=True)
                nc.scalar.activation(out_sb[:, b * 256:(b + 1) * 256], ps5[:],
                                     IDENT, bias=bt["b5"][:, 0:1])

            nc.sync.dma_start(out=out_d[:, :], in_=out_sb[:])

    nc.compile()
    return nc


def _get_compiled():
    global _compiled
    if _compiled is None:
        _compiled = _build_kernel()
    return _compiled


def kernel(photo, projection_points, params):
    global LAST_RESULT
    from concourse.bass_utils import run_bass_kernel_spmd

    feats = _host_features(photo, projection_points, params)  # [4, 8192, 1016]

    fcs = [(np.asarray(w, np.float32), np.asarray(b, np.float32))
           for (w, b) in params["fc"]]

    def lhsT(w, pad_in=None, pad_out=None, scale_rows=None):
        m = w.T.copy()  # [in, out]
        if scale_rows is not None:
            m[scale_rows[0]:scale_rows[1]] *= 0.25
        if pad_in:
            m = np.concatenate([m, np.zeros((pad_in - m.shape[0], m.shape[1]), m.dtype)], 0)
        if pad_out:
            m = np.concatenate([m, np.zeros((m.shape[0], pad_out - m.shape[1]), m.dtype)], 1)
        return m.astype(ml_dtypes.bfloat16)

    w0 = lhsT(fcs[0][0], pad_in=1024)
    w1 = lhsT(fcs[1][0])
    w2 = lhsT(fcs[2][0])
    w3 = lhsT(fcs[3][0], scale_rows=(1024, 2048))
    w4 = lhsT(fcs[4][0])
    w5 = lhsT(fcs[5][0], pad_out=32)

    def bias_tile(bv, npart=128):
        n = bv.shape[0]
        nch = max(1, (n + npart - 1) // npart)
        out = np.zeros((npart, nch), np.float32)
        for c in range(nch):
            seg = bv[c * npart:(c + 1) * npart]
            out[:len(seg), c] = seg
        return out

    ba = np.concatenate([bias_tile(fcs[0][1]), bias_tile(fcs[1][1]),
                         bias_tile(fcs[2][1]), bias_tile(fcs[3][1]),
                         bias_tile(fcs[4][1])], axis=1)  # [128, 21]
    b5 = bias_tile(fcs[5][1], npart=32)

    in_maps = []
    for c in range(N_CORES):
        a = feats[:, c * PTS_PER_CORE:(c + 1) * PTS_PER_CORE, :]   # [4,1024,1016]
        a = a.reshape(N_CAM, N_BLK, BLK_PTS, FEAT)
        a = a.transpose(3, 1, 0, 2).reshape(FEAT, COLS)            # [1016, 4096]
        x = np.zeros((FEAT_PAD, COLS), np.float32)
        x[:FEAT] = a
        in_maps.append({
            "x": x.astype(ml_dtypes.bfloat16),
            "w0": w0, "w1": w1, "w2": w2, "w3": w3, "w4": w4, "w5": w5,
            "ba": ba, "b5": b5,
        })

    nc = _get_compiled()
    res = run_bass_kernel_spmd(nc, in_maps, core_ids=list(range(N_CORES)))
    LAST_RESULT = res

    out = np.empty((N_PTS, 2), np.float32)
    for c in range(N_CORES):
        o = np.asarray(res.results[c]["out"], np.float32)  # [32, 1024]
        out[c * PTS_PER_CORE:(c + 1) * PTS_PER_CORE] = o[:2].T
    return out


# revision 9
# speedup vs baseline: 3.2101x; 3.2101x over previous
"""Trainium2 kernel for nn_Net_11312943858306.

Strategy: the FC head (~95% of FLOPs: 4x8192 points through
1016->512->512->1024, cross-camera max/mean reduce, 2048->512->128->2)
runs as a Bass/Tile SPMD kernel on 8 NeuronCores, data-parallel over
points (1024 points/core, all 4 cameras of a point on the same core ->
no collectives). The conv pyramid + bilinear gather (~5% of FLOPs,
data-dependent indexing) is prepared host-side in numpy.

Device layout: activations are [features-on-partitions, points-on-free];
weights are pre-transposed lhsT [in,out] bf16; matmuls accumulate fp32 in
PSUM; bias+relu fused on the scalar engine. Per core, points are split in
4 blocks of 256 (x 4 cams = 1024 columns) so SBUF holds weights + double
buffered activations.
"""

import sys

for _p in ("/opt/trn_rl_repo", "/root/.axon_site/_ro/trn_rl_repo"):
    if _p not in sys.path:
        sys.path.append(_p)

import numpy as np
import ml_dtypes

N_CAM = 4
N_PTS = 8192
IMG = 512
N_CORES = 8
PTS_PER_CORE = N_PTS // N_CORES          # 1024
BLK_PTS = 256                            # points per device block
N_BLK = PTS_PER_CORE // BLK_PTS          # 4
COLS = PTS_PER_CORE * N_CAM              # 4096
FEAT = 1016
FEAT_PAD = 1024

_compiled = None
LAST_RESULT = None
LAST_DISPATCH_S = None


# ----------------------------------------------------------------------
# Host-side reference-exact feature extraction (conv pyramid + gather)
# ----------------------------------------------------------------------

def _conv3x3(x, w, b):
    # x [N,C,H,W] fp32, w [O,I,3,3], pad 1
    n, c, h, ww = x.shape
    o = w.shape[0]
    xp = np.pad(x, ((0, 0), (0, 0), (1, 1), (1, 1)))
    win = np.lib.stride_tricks.sliding_window_view(xp, (3, 3), axis=(2, 3))
    # win [N,C,H,W,3,3] -> [N,H,W,C*9]
    win = win.transpose(0, 2, 3, 1, 4, 5).reshape(n * h * ww, c * 9)
    wm = w.reshape(o, c * 9).T.astype(np.float32)  # [C*9, O]
    y = win.astype(np.float32) @ wm
    y = y.reshape(n, h, ww, o).transpose(0, 3, 1, 2)
    return y + b[None, :, None, None]


def _bn(x, gamma, beta, eps=1e-5):
    mu = x.mean(axis=(0, 2, 3), keepdims=True, dtype=np.float32)
    var = ((x - mu) ** 2).mean(axis=(0, 2, 3), keepdims=True, dtype=np.float32)
    xhat = (x - mu) / np.sqrt(var + eps)
    return gamma[None, :, None, None] * xhat + beta[None, :, None, None]


def _maxpool2(x):
    n, c, h, w = x.shape
    return x.reshape(n, c, h // 2, 2, w // 2, 2).max(axis=(3, 5))


def _bilinear_gather(x, px, py):
    H, W = x.shape[2], x.shape[3]
    px = px.astype(np.float32)
    py = py.astype(np.float32)
    xc = np.ceil(px)
    xf = xc - 1.0
    yc = np.ceil(py)
    yf = yc - 1.0
    inside = ((px <= H - 1) & (px >= 0) & (py <= W - 1) & (py >= 0)).astype(px.dtype)
    xc, xf, yc, yf = xc * inside, xf * inside, yc * inside, yf * inside
    cam = np.arange(x.shape[0])[:, None]
    xic, xif = xc.astype(np.int32), xf.astype(np.int32)
    yic, yif = yc.astype(np.int32), yf.astype(np.int32)
    v_cc = x[cam, :, xic, yic]
    v_fc = x[cam, :, xif, yic]
    v_cf = x[cam, :, xic, yif]
    v_ff = x[cam, :, xif, yif]
    w_cc = ((xf - px) * (yf - py))[..., None]
    w_fc = ((px - xc) * (yf - py))[..., None]
    w_cf = ((xf - px) * (py - yc))[..., None]
    w_ff = ((px - xc) * (py - yc))[..., None]
    return v_cc * w_cc + v_fc * w_fc + v_cf * w_cf + v_ff * w_ff


def _host_features(photo, projection_points, params):
    convs = [(np.asarray(w, np.float32), np.asarray(b, np.float32))
             for (w, b) in params["conv"]]
    bns = [(np.asarray(g, np.float32), np.asarray(b, np.float32))
           for (g, b) in params["bn"]]
    px = np.asarray(projection_points[:, :, 0], np.float32)
    py = np.asarray(projection_points[:, :, 1], np.float32)
    feats = []
    x = _bn(_conv3x3(np.asarray(photo, np.float32), *convs[0]), *bns[0])
    feats.append(_bilinear_gather(x, px, py))
    px, py = px / 2, py / 2
    ci = 1
    for _ in range(6):
        x = _maxpool2(x)
        x = np.maximum(_bn(_conv3x3(x, *convs[ci]), *bns[ci]), 0.0); ci += 1
        x = np.maximum(_bn(_conv3x3(x, *convs[ci]), *bns[ci]), 0.0); ci += 1
        feats.append(_bilinear_gather(x, px, py))
        px, py = px / 2, py / 2
    return np.concatenate(feats, axis=-1).astype(np.float32)  # [cam, N, 1016]


# ----------------------------------------------------------------------
# Device kernel (Bass/Tile), SPMD over 8 cores
# ----------------------------------------------------------------------

def _build_kernel():
    import concourse.bass as bass
    import concourse.bacc as bacc
    import concourse.mybir as mybir
    import concourse.tile as tile

    bf16 = mybir.dt.bfloat16
    f32 = mybir.dt.float32
    RELU = mybir.ActivationFunctionType.Relu
    IDENT = mybir.ActivationFunctionType.Identity
    MAX = mybir.AluOpType.max

    nc = bacc.Bacc()
    x_d = nc.dram_tensor("x", [FEAT_PAD, COLS], bf16, kind="ExternalInput")
    w0_d = nc.dram_tensor("w0", [1024, 512], bf16, kind="ExternalInput")
    w1_d = nc.dram_tensor("w1", [512, 512], bf16, kind="ExternalInput")
    w2_d = nc.dram_tensor("w2", [512, 1024], bf16, kind="ExternalInput")
    w3_d = nc.dram_tensor("w3", [2048, 512], bf16, kind="ExternalInput")
    w4_d = nc.dram_tensor("w4", [512, 128], bf16, kind="ExternalInput")
    w5_d = nc.dram_tensor("w5", [128, 32], bf16, kind="ExternalInput")
    ba_d = nc.dram_tensor("ba", [128, 21], f32, kind="ExternalInput")
    b5_d = nc.dram_tensor("b5", [32, 1], f32, kind="ExternalInput")
    out_d = nc.dram_tensor("out", [32, COLS // N_CAM], f32, kind="ExternalOutput")

    x_r = x_d[:, :].rearrange("(kc p) n -> kc p n", p=128)          # [8,128,4096]
    w_r = {
        0: w0_d[:, :].rearrange("(kc p) m -> kc p m", p=128),        # [8,128,512]
        1: w1_d[:, :].rearrange("(kc p) m -> kc p m", p=128),        # [4,128,512]
        2: w2_d[:, :].rearrange("(kc p) m -> kc p m", p=128),        # [4,128,1024]
        3: w3_d[:, :].rearrange("(kc p) m -> kc p m", p=128),        # [16,128,512]
        4: w4_d[:, :].rearrange("(kc p) m -> kc p m", p=128),        # [4,128,128]
    }

    with tile.TileContext(nc) as tc:
        with (
            tc.tile_pool(name="wpool", bufs=1) as wp,
            tc.tile_pool(name="xpool", bufs=2) as xp,
            tc.tile_pool(name="hpool", bufs=2) as hp,
            tc.tile_pool(name="opool", bufs=1) as op,
            tc.tile_pool(name="psum", bufs=2, space="PSUM") as pp,
        ):
            # resident weights/biases (one DMA per tensor)
            wdims = {0: (8, 512), 1: (4, 512), 2: (4, 1024), 3: (16, 512), 4: (4, 128)}
            wsrc = {0: w0_d, 1: w1_d, 2: w2_d, 3: w3_d, 4: w4_d}
            wt = {}
            for li, (nk, m) in wdims.items():
                t = wp.tile([128, nk * m], bf16, tag=f"wl{li}", name=f"wl{li}")
                nc.sync.dma_start(
                    out=t[:].rearrange("p (kc m) -> p kc m", m=m),
                    in_=wsrc[li][:, :].rearrange("(kc p) m -> p kc m", p=128))
                wt[li] = [t[:, k * m:(k + 1) * m] for k in range(nk)]
            w5t = wp.tile([128, 32], bf16, tag="w5")
            nc.sync.dma_start(out=w5t[:], in_=w5_d[:, :])
            bat = wp.tile([128, 21], f32, tag="ba")
            nc.sync.dma_start(out=bat[:], in_=ba_d[:, :])
            b5t = wp.tile([32, 1], f32, tag="b5t")
            nc.sync.dma_start(out=b5t[:], in_=b5_d[:, :])
            bt = {"b0": bat[:, 0:4], "b1": bat[:, 4:8], "b2": bat[:, 8:16],
                  "b3": bat[:, 16:20], "b4": bat[:, 20:21], "b5": b5t}

            out_sb = op.tile([32, COLS // N_CAM], f32, tag="out_sb")

            for b in range(N_BLK):
                c0 = b * 1024
                # load x block: one DMA, 8 K-chunks side by side
                xbt = xp.tile([128, 8192], bf16, tag="xbt", name="xbt")
                nc.sync.dma_start(
                    out=xbt[:].rearrange("p (kc n) -> p kc n", n=1024),
                    in_=x_d[:, :].rearrange("(kc p) n -> p kc n", p=128)[:, :, c0:c0 + 1024])
                xb = [xbt[:, k * 1024:(k + 1) * 1024] for k in range(8)]

                # fc0: 1024 -> 512, relu
                h0 = [hp.tile([128, 1024], bf16, tag=f"h0_{m}", name=f"h0_{m}") for m in range(4)]
                for m in range(4):
                    for nt in range(2):
                        ps = pp.tile([128, 512], f32, tag="ps_a")
                        for k in range(8):
                            nc.tensor.matmul(ps[:], wt[0][k][:, m * 128:(m + 1) * 128],
                                             xb[k][:, nt * 512:(nt + 1) * 512],
                                             start=(k == 0), stop=(k == 7))
                        nc.scalar.activation(h0[m][:, nt * 512:(nt + 1) * 512], ps[:],
                                             RELU, bias=bt["b0"][:, m:m + 1])

                # fc1: 512 -> 512, relu
                h1 = [hp.tile([128, 1024], bf16, tag=f"h1_{m}", name=f"h1_{m}") for m in range(4)]
                for m in range(4):
                    for nt in range(2):
                        ps = pp.tile([128, 512], f32, tag="ps_a")
                        for k in range(4):
                            nc.tensor.matmul(ps[:], wt[1][k][:, m * 128:(m + 1) * 128],
                                             h0[k][:, nt * 512:(nt + 1) * 512],
                                             start=(k == 0), stop=(k == 3))
                        nc.scalar.activation(h1[m][:, nt * 512:(nt + 1) * 512], ps[:],
                                             RELU, bias=bt["b1"][:, m:m + 1])

                # fc2: 512 -> 1024, no relu
                h2 = [hp.tile([128, 1024], bf16, tag=f"h2_{m}", name=f"h2_{m}") for m in range(8)]
                for m in range(8):
                    for nt in range(2):
                        ps = pp.tile([128, 512], f32, tag="ps_a")
                        for k in range(4):
                            nc.tensor.matmul(ps[:], wt[2][k][:, m * 128:(m + 1) * 128],
                                             h1[k][:, nt * 512:(nt + 1) * 512],
                                             start=(k == 0), stop=(k == 3))
                        nc.scalar.activation(h2[m][:, nt * 512:(nt + 1) * 512], ps[:],
                                             IDENT, bias=bt["b2"][:, m:m + 1])

                # cross-camera reduce: cols are [cam0|cam1|cam2|cam3] x 256 pts
                zc = []
                for m in range(8):
                    zm = hp.tile([128, 256], bf16, tag=f"zmx_{m}", name=f"zmx_{m}")
                    t01 = hp.tile([128, 256], bf16, tag=f"t01_{m}", name=f"t01_{m}")
                    nc.vector.tensor_tensor(out=t01[:], in0=h2[m][:, 0:256],
                                            in1=h2[m][:, 256:512], op=MAX)
                    nc.vector.tensor_tensor(out=zm[:], in0=h2[m][:, 512:768],
                                            in1=h2[m][:, 768:1024], op=MAX)
                    nc.vector.tensor_tensor(out=zm[:], in0=zm[:], in1=t01[:], op=MAX)
                    zc.append(zm)
                for m in range(8):
                    zs = hp.tile([128, 256], bf16, tag=f"zsm_{m}", name=f"zsm_{m}")
                    t01 = hp.tile([128, 256], bf16, tag=f"s01_{m}", name=f"s01_{m}")
                    nc.vector.tensor_add(out=t01[:], in0=h2[m][:, 0:256],
                                         in1=h2[m][:, 256:512])
                    nc.vector.tensor_add(out=zs[:], in0=h2[m][:, 512:768],
                                         in1=h2[m][:, 768:1024])
                    nc.vector.tensor_add(out=zs[:], in0=zs[:], in1=t01[:])
                    zc.append(zs)  # mean fold (x0.25) is baked into w3 rows

                # fc3: 2048 -> 512, relu   (N = 256)
                h3 = [hp.tile([128, 256], bf16, tag=f"h3_{m}", name=f"h3_{m}") for m in range(4)]
                for m in range(4):
                    ps = pp.tile([128, 256], f32, tag="ps_b")
                    for k in range(16):
                        nc.tensor.matmul(ps[:], wt[3][k][:, m * 128:(m + 1) * 128],
                                         zc[k][:], start=(k == 0), stop=(k == 15))
                    nc.scalar.activation(h3[m][:], ps[:], RELU,
                                         bias=bt["b3"][:, m:m + 1])

                # fc4: 512 -> 128, relu
                h4 = hp.tile([128, 256], bf16, tag="h4")
                ps = pp.tile([128, 256], f32, tag="ps_b")
                for k in range(4):
                    nc.tensor.matmul(ps[:], wt[4][k], h3[k][:],
                                     start=(k == 0), stop=(k == 3))
                nc.scalar.activation(h4[:], ps[:], RELU, bias=bt["b4"][:, 0:1])

                # fc5: 128 -> 2 (padded to 32)
                ps5 = pp.tile([32, 256], f32, tag="ps_c")
                nc.tensor.matmul(ps5[:], w5t[:], h4[:], start=True, stop=True)
                nc.scalar.activation(out_sb[:, b * 256:(b + 1) * 256], ps5[:],
                                     IDENT, bias=bt["b5"][:, 0:1])

            nc.sync.dma_start(out=out_d[:, :], in_=out_sb[:])

    nc.compile()
    return nc


def _get_compiled():
    global _compiled
    if _compiled is None:
        _compiled = _build_kernel()
    return _compiled


def kernel(photo, projection_points, params):
    global LAST_RESULT
    from concourse.bass_utils import run_bass_kernel_spmd

    feats = _host_features(photo, projection_points, params)  # [4, 8192, 1016]

    fcs = [(np.asarray(w, np.float32), np.asarray(b, np.float32))
           for (w, b) in params["fc"]]

    def lhsT(w, pad_in=None, pad_out=None, scale_rows=None):
        m = w.T.copy()  # [in, out]
        if scale_rows is not None:
            m[scale_rows[0]:scale_rows[1]] *= 0.25
        if pad_in:
            m = np.concatenate([m, np.zeros((pad_in - m.shape[0], m.shape[1]), m.dtype)], 0)
        if pad_out:
            m = np.concatenate([m, np.zeros((m.shape[0], pad_out - m.shape[1]), m.dtype)], 1)
        return m.astype(ml_dtypes.bfloat16)

    w0 = lhsT(fcs[0][0], pad_in=1024)
    w1 = lhsT(fcs[1][0])
    w2 = lhsT(fcs[2][0])
    w3 = lhsT(fcs[3][0], scale_rows=(1024, 2048))
    w4 = lhsT(fcs[4][0])
    w5 = lhsT(fcs[5][0], pad_out=32)

    def bias_tile(bv, npart=128):
        n = bv.shape[0]
        nch = max(1, (n + npart - 1) // npart)
        out = np.zeros((npart, nch), np.float32)
        for c in range(nch):
            seg = bv[c * npart:(c + 1) * npart]
            out[:len(seg), c] = seg
        return out

    ba = np.concatenate([bias_tile(fcs[0][1]), bias_tile(fcs[1][1]),
                         bias_tile(fcs[2][1]), bias_tile(fcs[3][1]),
                         bias_tile(fcs[4][1])], axis=1)  # [128, 21]
    b5 = bias_tile(fcs[5][1], npart=32)

    in_maps = []
    for c in range(N_CORES):
        a = feats[:, c * PTS_PER_CORE:(c + 1) * PTS_PER_CORE, :]   # [4,1024,1016]
        a = a.reshape(N_CAM, N_BLK, BLK_PTS, FEAT)
        a = a.transpose(3, 1, 0, 2).reshape(FEAT, COLS)            # [1016, 4096]
        x = np.zeros((FEAT_PAD, COLS), np.float32)
        x[:FEAT] = a
        in_maps.append({
            "x": x.astype(ml_dtypes.bfloat16),
            "w0": w0, "w1": w1, "w2": w2, "w3": w3, "w4": w4, "w5": w5,
            "ba": ba, "b5": b5,
        })

    import os
    import time as _time
    global LAST_DISPATCH_S
    nc = _get_compiled()
    _trace = bool(os.environ.get("KTRACE"))
    _t0 = _time.time()
    res = run_bass_kernel_spmd(nc, in_maps, core_ids=list(range(N_CORES)),
                               trace=_trace)
    LAST_DISPATCH_S = _time.time() - _t0
    LAST_RESULT = res

    out = np.empty((N_PTS, 2), np.float32)
    for c in range(N_CORES):
        o = np.asarray(res.results[c]["out"], np.float32)  # [32, 1024]
        out[c * PTS_PER_CORE:(c + 1) * PTS_PER_CORE] = o[:2].T
    return out
